# revision 23
# baseline (speedup 1.0000x reference)
"""Trainium2 Bass kernel for a 4-layer binary MLP (BinaryFCNN).

Reference computation (per layer):  h = sign_pm1(h @ sign_pm1(W).T + b)
with x: [8192, 4096] fp32, W_l: [4096, 4096] fp32, b_l: [4096] fp32.

Measured on 8 axon-tunneled TRN2 NeuronCores: HW exec time ~1.61 ms/core,
relative error 0.0092 vs the fp32 jax reference (a single borderline sign
flip in layer 1 out of 33.5M elements; binary nets amplify any flip, and
~1 flip is the irreducible level for any non-bit-identical fp32 matmul --
fp64 numpy vs jax-fp32-on-neuron measured 0 flips, so the reference itself
sits in that regime).

Strategy
--------
* Data-parallel over the batch: core c processes rows [c*1024, (c+1)*1024).
  No collectives; every core streams all four weight matrices (bf16, 33.5 MB
  per layer) -- fully hidden under the matmul stream.
* Activations live in SBUF feature-major ("h.T" layout, [feat, batch]): each
  layer's output is directly the next layer's moving operand; no transposes.
* All arithmetic on device. Host does value-preserving marshaling only:
  transpose/pack, fp32->bf16 cast of W (sign-preserving at these magnitudes),
  final bf16 -> fp32 cast of the +-1 output.
* Weights are encoded on device as (w >= 0) - 0.5 in {-0.5, +0.5} (one DVE
  op). +-0.5 is a power of two so products/partial sums stay exact; the sign
  activation uses ACT Sign(2*psum + b) == sign(h @ sign(W).T + b), with the
  per-feature bias as the ACT per-partition bias operand -- one ACT op per
  PSUM tile, reading PSUM and writing the next h tile.
* Layers 2..4 are bit-exact: +-1 activations and +-0.5 weights in fp8e4m3
  with fp32 PSUM accumulation (half-integer sums << 2^24). fp8 DoubleRow
  (perf_mode) pairs k-chunks for 2 fp8 MACs/PE/cycle -- these layers run at
  2x bf16 rate (~218 us each).
* Layer 1 is the only inexact matmul. x is split on device into two fp16
  digits: hi = fp16(x), lo = fp16((x - hi) * 2^11) (scaling dodges fp16
  subnormal flush; the lo pass uses weights pre-scaled by 2^-11, an exact
  power-of-two). Both digit passes accumulate into one PSUM tile. fp16 is
  the optimal digit dtype: 11 mantissa bits per PE pass vs 8 (bf16) or
  2x4 (fp8 DoubleRow pair).
* A short burst of throwaway matmuls during the x-prep prologue releases the
  PE HAM clock gate (cold 1.2 GHz -> warm 2.4 GHz) before the real stream.

Per-core floor: layer 1 = 2 fp16 passes = 874 us, layers 2-4 = 3 DR passes
= 654 us; measured 1.61 ms = floor + ~5% (prologue, mh boundary, tail drain).
"""
import numpy as np
import ml_dtypes

import concourse.bass as bass
import concourse.tile as tile
from concourse import bacc
import concourse.mybir as mybir
from concourse.bass_utils import run_bass_kernel_spmd

F32 = mybir.dt.float32
F16 = mybir.dt.float16
BF16 = mybir.dt.bfloat16
FP8 = mybir.dt.float8e4
ALU = mybir.AluOpType
SIGN = mybir.ActivationFunctionType.Sign

N_CORES = 8
D_FULL = 4096
B_FULL = 8192
MF = 512  # matmul moving free dim == one fp32 PSUM bank

# fp8 DoubleRow for layers 2..4: 2 fp8 MACs/PE/cycle (measured ~1.4x at FD=512),
# and halves the matmul instruction count. Exactness is preserved (+-1 x +-0.5
# products, fp32 accumulate).
USE_DOUBLE_ROW = True


def build_binary_mlp(D: int, M: int, n_layers: int = 4) -> bass.Bass:
    """Emit the per-core kernel. D = feature dim, M = per-core batch rows."""
    KO = D // 128  # contraction chunks (also input-feature blocks)
    NB = D // 128  # output-feature blocks
    MF = min(512, M)  # moving free dim (one fp32 PSUM bank at 512)
    MH = M // MF   # batch slices of the moving operand

    # Bacc (not raw Bass) + the trailing compile() pass: Bacc's compile
    # legalizes things raw Bass output trips over in walrus (e.g. the
    # 1-sync-wait-per-instruction cap).
    nc = bacc.Bacc("TRN2", target_bir_lowering=False, debug=False)
    xt = nc.declare_dram_parameter("xt", [D, M], F32, isOutput=False)
    ws = [
        nc.declare_dram_parameter(f"w{l + 1}", [NB, 128, KO, 128], BF16, isOutput=False)
        for l in range(n_layers)
    ]
    bs = [
        nc.declare_dram_parameter(f"b{l + 1}", [128, NB], F32, isOutput=False)
        for l in range(n_layers)
    ]
    out = nc.declare_dram_parameter("out", [NB, 128, M], BF16, isOutput=True)

    with tile.TileContext(nc) as tc:
        with (
            tc.tile_pool(name="const", bufs=1) as const,
            tc.tile_pool(name="wraw", bufs=3) as wraw,
            tc.tile_pool(name="wsgn", bufs=2) as wsgn,
            tc.tile_pool(name="xio", bufs=4) as xio,
            tc.tile_pool(name="psum", bufs=5, space="PSUM") as psum,
            tc.tile_pool(name="psum1", bufs=1, space="PSUM") as psum1,
        ):
            bias_tiles = []
            for l in range(n_layers):
                bt = const.tile([128, NB], F32, tag=f"bias{l}", name=f"bias{l}")
                nc.sync.dma_start(bt[:], bs[l][:])
                bias_tiles.append(bt)

            # PE warm-up: the first real matmul waits ~20us for the x digit
            # prep; fill that window with throwaway matmuls so the HAM clock
            # gate (cold 1.2 GHz -> warm 2.4 GHz after ~3.4us of activity) is
            # already released when the real stream starts.
            if M >= 512:
                wu = const.tile([128, MF], F16, tag="warm", name="warm")
                nc.vector.memset(wu[:], 1.0)
                wps = psum1.tile([128, MF], F32, tag="wps", name="wps")
                n_wu = 72
                for i in range(n_wu):
                    nc.tensor.matmul(wps[:], wu[:, :128], wu[:],
                                     start=(i == 0), stop=(i == n_wu - 1))

            # ping-pong activation buffers, feature-major, +-1 in fp8
            hA = const.tile([128, KO, M], FP8, tag="hA", name="hA")
            hB = const.tile([128, KO, M], FP8, tag="hB", name="hB")

            # ---------------- layer 1: fp16 hi/lo digit passes ----------------
            for mh in range(MH):  # noqa: B007
                ms = slice(mh * MF, (mh + 1) * MF)
                hi = const.tile([128, KO, MF], F16, tag="hi", name="hi")
                lo = const.tile([128, KO, MF], F16, tag="lo", name="lo")
                KC = 2  # k-chunks per DMA: each DMA trigger costs ~0.65us of
                # sync-sequencer issue time, so fewer+bigger transfers shorten
                # the (sequencer-paced) digit-prep windows
                for kg in range(0, KO, KC):
                    xc = xio.tile([128, KC, MF], F32, tag="xc", name="xc", bufs=2)
                    nc.sync.dma_start(
                        xc[:],
                        xt[kg * 128:(kg + KC) * 128, ms].rearrange(
                            "(t p) m -> p t m", p=128))
                    nc.scalar.copy(hi[:, kg:kg + KC, :], xc[:])  # fp16 RNE
                    r = xio.tile([128, KC, MF], F32, tag="r", name="r", bufs=2)
                    nc.vector.tensor_sub(r[:], xc[:], hi[:, kg:kg + KC, :])
                    # scale residual by 2^11 so fp16 never goes subnormal
                    nc.vector.tensor_scalar_mul(lo[:, kg:kg + KC, :], r[:], 2048.0)
                for nb in range(NB):
                    wt = wraw.tile([128, KO, 128], BF16, tag="w", name="wt")
                    nc.sync.dma_start(wt[:], ws[0][nb])
                    sw_hi = wsgn.tile([128, KO, 128], F16, tag="swhi", name="sw_hi")
                    nc.vector.tensor_scalar(sw_hi[:], wt[:], 0.0, 0.5, ALU.is_ge, ALU.subtract)
                    sw_lo = wsgn.tile([128, KO, 128], F16, tag="swlo", name="sw_lo")
                    nc.vector.tensor_scalar_mul(sw_lo[:], sw_hi[:], 2.0 ** -11)
                    ps = psum.tile([128, MF], F32, tag="ps", name="ps")
                    for ko in range(KO):
                        nc.tensor.matmul(ps[:], sw_hi[:, ko, :], hi[:, ko, :],
                                         start=(ko == 0), stop=False)
                        nc.tensor.matmul(ps[:], sw_lo[:, ko, :], lo[:, ko, :],
                                         start=False, stop=(ko == KO - 1))
                    # h1 = Sign(2*psum + b) in {-1, +1} (psum = 0.5 * x@sign(W).T)
                    nc.scalar.activation(hA[:, nb, ms], ps[:], SIGN,
                                         bias=bias_tiles[0][:, nb:nb + 1], scale=2.0)

            # ---------------- layers 2..n: exact +-1 x +-0.5 ----------------
            hin, hout = hA, hB
            for l in range(1, n_layers):
                last = l == n_layers - 1
                for nb in range(NB):
                    wt = wraw.tile([128, KO, 128], BF16, tag="w", name="wt")
                    nc.sync.dma_start(wt[:], ws[l][nb])
                    sw = wsgn.tile([128, KO, 128], FP8, tag="swhi", name="sw")
                    nc.vector.tensor_scalar(sw[:], wt[:], 0.0, 0.5, ALU.is_ge, ALU.subtract)
                    for mh in range(MH):
                        ms = slice(mh * MF, (mh + 1) * MF)
                        ps = psum.tile([128, MF], F32, tag="ps", name="ps")
                        if USE_DOUBLE_ROW and KO % 2 == 0:
                            for ko in range(0, KO, 2):
                                nc.tensor.matmul(
                                    ps[:], sw[:, ko:ko + 2, :], hin[:, ko:ko + 2, ms],
                                    start=(ko == 0), stop=(ko + 2 == KO),
                                    perf_mode=mybir.MatmulPerfMode.DoubleRow)
                        else:
                            for ko in range(KO):
                                nc.tensor.matmul(ps[:], sw[:, ko, :], hin[:, ko, ms],
                                                 start=(ko == 0), stop=(ko == KO - 1))
                        if last:
                            ot = xio.tile([128, MF], BF16, tag="ot", name="ot")
                            nc.scalar.activation(ot[:], ps[:], SIGN,
                                                 bias=bias_tiles[l][:, nb:nb + 1], scale=2.0)
                            nc.sync.dma_start(out[nb, :, ms], ot[:])
                        else:
                            nc.scalar.activation(hout[:, nb, ms], ps[:], SIGN,
                                                 bias=bias_tiles[l][:, nb:nb + 1], scale=2.0)
                hin, hout = hout, hin
    nc.compile()
    return nc


def _pack_w(W: np.ndarray) -> np.ndarray:
    """W [D, D] fp32 -> [NB, 128(p=k_in), KO, 128(n)] bf16 with
    WP[nb, p, ko, n] = bf16(W[nb*128 + n, ko*128 + p]).  Pure layout + a
    sign-preserving cast (bf16 keeps fp32's exponent range, so no value here
    can round across or onto zero)."""
    D = W.shape[0]
    nb = D // 128
    return np.ascontiguousarray(
        W.astype(ml_dtypes.bfloat16).reshape(nb, 128, nb, 128).transpose(0, 3, 2, 1)
    )


def _pack_b(b: np.ndarray) -> np.ndarray:
    return np.ascontiguousarray(b.astype(np.float32).reshape(-1, 128).T)


last_result = None  # BassKernelResults of the most recent run (for test.py)
_nc_cache = {}


def kernel(x, W1, b1, W2, b2, W3, b3, W4, b4):
    global last_result
    assert x.shape == (B_FULL, D_FULL)
    M = B_FULL // N_CORES

    if (D_FULL, M) not in _nc_cache:
        _nc_cache[(D_FULL, M)] = build_binary_mlp(D_FULL, M)
    nc = _nc_cache[(D_FULL, M)]

    xt = x.astype(np.float32).T  # [D, B]
    shared = {}
    for l, (W, b) in enumerate(((W1, b1), (W2, b2), (W3, b3), (W4, b4)), start=1):
        shared[f"w{l}"] = _pack_w(np.asarray(W))
        shared[f"b{l}"] = _pack_b(np.asarray(b))

    in_maps = []
    for c in range(N_CORES):
        m = dict(shared)
        m["xt"] = np.ascontiguousarray(xt[:, c * M:(c + 1) * M])
        in_maps.append(m)

    try:
        res = run_bass_kernel_spmd(nc, in_maps, core_ids=list(range(N_CORES)))
    except Exception:
        # one retry for transient device hiccups (NRT_EXEC_UNIT_UNRECOVERABLE
        # was observed once on an otherwise healthy worker)
        res = run_bass_kernel_spmd(nc, in_maps, core_ids=list(range(N_CORES)))
    last_result = res

    parts = []
    for c in range(N_CORES):
        o = np.asarray(res.results[c]["out"])  # [NB, 128, M] bf16, values +-1
        parts.append(o.reshape(D_FULL, M).T)   # -> [M, D] (rows are batch)
    return np.concatenate(parts, axis=0).astype(np.float32)


# revision 24
# speedup vs baseline: 1.2058x; 1.2058x over previous
"""Trainium2 Bass kernel for a 4-layer binary MLP (BinaryFCNN).

Reference computation (per layer):  h = sign_pm1(h @ sign_pm1(W).T + b)
with x: [8192, 4096] fp32, W_l: [4096, 4096] fp32, b_l: [4096] fp32.

Measured on 8 axon-tunneled TRN2 NeuronCores: HW exec time ~1.61 ms/core,
relative error 0.0092 vs the fp32 jax reference (a single borderline sign
flip in layer 1 out of 33.5M elements; binary nets amplify any flip, and
~1 flip is the irreducible level for any non-bit-identical fp32 matmul --
fp64 numpy vs jax-fp32-on-neuron measured 0 flips, so the reference itself
sits in that regime).

Strategy
--------
* Data-parallel over the batch: core c processes rows [c*1024, (c+1)*1024).
  No collectives; every core streams all four weight matrices (bf16, 33.5 MB
  per layer) -- fully hidden under the matmul stream.
* Activations live in SBUF feature-major ("h.T" layout, [feat, batch]): each
  layer's output is directly the next layer's moving operand; no transposes.
* All arithmetic on device. Host does value-preserving marshaling only:
  transpose/pack, fp32->bf16 cast of W (sign-preserving at these magnitudes),
  final bf16 -> fp32 cast of the +-1 output.
* Weights are encoded on device as (w >= 0) - 0.5 in {-0.5, +0.5} (one DVE
  op). +-0.5 is a power of two so products/partial sums stay exact; the sign
  activation uses ACT Sign(2*psum + b) == sign(h @ sign(W).T + b), with the
  per-feature bias as the ACT per-partition bias operand -- one ACT op per
  PSUM tile, reading PSUM and writing the next h tile.
* Layers 2..4 are bit-exact: +-1 activations and +-0.5 weights in fp8e4m3
  with fp32 PSUM accumulation (half-integer sums << 2^24). fp8 DoubleRow
  (perf_mode) pairs k-chunks for 2 fp8 MACs/PE/cycle -- these layers run at
  2x bf16 rate (~218 us each).
* Layer 1 is the only inexact matmul. x is split on device into two fp16
  digits: hi = fp16(x), lo = fp16((x - hi) * 2^11) (scaling dodges fp16
  subnormal flush; the lo pass uses weights pre-scaled by 2^-11, an exact
  power-of-two). Both digit passes accumulate into one PSUM tile. fp16 is
  the optimal digit dtype: 11 mantissa bits per PE pass vs 8 (bf16) or
  2x4 (fp8 DoubleRow pair).
* A short burst of throwaway matmuls during the x-prep prologue releases the
  PE HAM clock gate (cold 1.2 GHz -> warm 2.4 GHz) before the real stream.

Per-core floor: layer 1 = 2 fp16 passes = 874 us, layers 2-4 = 3 DR passes
= 654 us; measured 1.61 ms = floor + ~5% (prologue, mh boundary, tail drain).
"""
import numpy as np
import ml_dtypes

import concourse.bass as bass
import concourse.tile as tile
from concourse import bacc
import concourse.mybir as mybir
from concourse.bass_utils import run_bass_kernel_spmd

F32 = mybir.dt.float32
F16 = mybir.dt.float16
BF16 = mybir.dt.bfloat16
FP8 = mybir.dt.float8e4
ALU = mybir.AluOpType
SIGN = mybir.ActivationFunctionType.Sign

N_CORES = 8
D_FULL = 4096
B_FULL = 8192
MF = 512  # matmul moving free dim == one fp32 PSUM bank

# fp8 DoubleRow for layers 2..4: 2 fp8 MACs/PE/cycle (measured ~1.4x at FD=512),
# and halves the matmul instruction count. Exactness is preserved (+-1 x +-0.5
# products, fp32 accumulate).
USE_DOUBLE_ROW = True


def build_binary_mlp(D: int, M: int, n_layers: int = 4) -> bass.Bass:
    """Emit the per-core kernel. D = feature dim, M = per-core batch rows."""
    KO = D // 128  # contraction chunks (also input-feature blocks)
    NB = D // 128  # output-feature blocks
    MF = min(512, M)  # moving free dim (one fp32 PSUM bank at 512)
    MH = M // MF   # batch slices of the moving operand

    # Bacc (not raw Bass) + the trailing compile() pass: Bacc's compile
    # legalizes things raw Bass output trips over in walrus (e.g. the
    # 1-sync-wait-per-instruction cap).
    nc = bacc.Bacc("TRN2", target_bir_lowering=False, debug=False)
    xt = nc.declare_dram_parameter("xt", [D, M], F32, isOutput=False)
    ws = [
        nc.declare_dram_parameter(f"w{l + 1}", [NB, 128, KO, 128], BF16, isOutput=False)
        for l in range(n_layers)
    ]
    bs = [
        nc.declare_dram_parameter(f"b{l + 1}", [128, NB], F32, isOutput=False)
        for l in range(n_layers)
    ]
    out = nc.declare_dram_parameter("out", [NB, 128, M], BF16, isOutput=True)

    with tile.TileContext(nc) as tc:
        with (
            tc.tile_pool(name="const", bufs=1) as const,
            tc.tile_pool(name="wraw", bufs=3) as wraw,
            tc.tile_pool(name="wsgn", bufs=2) as wsgn,
            tc.tile_pool(name="xio", bufs=4) as xio,
            tc.tile_pool(name="psum", bufs=5, space="PSUM") as psum,
            tc.tile_pool(name="psum1", bufs=1, space="PSUM") as psum1,
        ):
            bias_tiles = []
            for l in range(n_layers):
                bt = const.tile([128, NB], F32, tag=f"bias{l}", name=f"bias{l}")
                nc.sync.dma_start(bt[:], bs[l][:])
                bias_tiles.append(bt)

            # PE warm-up: the first real matmul waits ~20us for the x digit
            # prep; fill that window with throwaway matmuls so the HAM clock
            # gate (cold 1.2 GHz -> warm 2.4 GHz after ~3.4us of activity) is
            # already released when the real stream starts.
            if M >= 512:
                wu = const.tile([128, MF], F16, tag="warm", name="warm")
                nc.vector.memset(wu[:], 1.0)
                wps = psum1.tile([128, MF], F32, tag="wps", name="wps")
                n_wu = 72
                for i in range(n_wu):
                    nc.tensor.matmul(wps[:], wu[:, :128], wu[:],
                                     start=(i == 0), stop=(i == n_wu - 1))

            # ping-pong activation buffers, feature-major, +-1 in fp8
            hA = const.tile([128, KO, M], FP8, tag="hA", name="hA")
            hB = const.tile([128, KO, M], FP8, tag="hB", name="hB")

            # ---------------- layer 1: fp16 hi/lo digit passes ----------------
            for mh in range(MH):  # noqa: B007
                ms = slice(mh * MF, (mh + 1) * MF)
                hi = const.tile([128, KO, MF], F16, tag="hi", name="hi")
                lo = const.tile([128, KO, MF], F16, tag="lo", name="lo")
                for ko in range(KO):
                    xc = xio.tile([128, MF], F32, tag="xc", name="xc")
                    nc.sync.dma_start(xc[:], xt[ko * 128:(ko + 1) * 128, ms])
                    nc.scalar.copy(hi[:, ko, :], xc[:])  # fp16 round-to-nearest
                    r = xio.tile([128, MF], F32, tag="r", name="r")
                    nc.vector.tensor_sub(r[:], xc[:], hi[:, ko, :])
                    # scale residual by 2^11 so fp16 never goes subnormal
                    nc.vector.tensor_scalar_mul(lo[:, ko, :], r[:], 2048.0)
                for nb in range(NB):
                    wt = wraw.tile([128, KO, 128], BF16, tag="w", name="wt")
                    nc.sync.dma_start(wt[:], ws[0][nb])
                    sw_hi = wsgn.tile([128, KO, 128], F16, tag="swhi", name="sw_hi")
                    nc.vector.tensor_scalar(sw_hi[:], wt[:], 0.0, 0.5, ALU.is_ge, ALU.subtract)
                    sw_lo = wsgn.tile([128, KO, 128], F16, tag="swlo", name="sw_lo")
                    nc.vector.tensor_scalar_mul(sw_lo[:], sw_hi[:], 2.0 ** -11)
                    ps = psum.tile([128, MF], F32, tag="ps", name="ps")
                    for ko in range(KO):
                        nc.tensor.matmul(ps[:], sw_hi[:, ko, :], hi[:, ko, :],
                                         start=(ko == 0), stop=False)
                        nc.tensor.matmul(ps[:], sw_lo[:, ko, :], lo[:, ko, :],
                                         start=False, stop=(ko == KO - 1))
                    # h1 = Sign(2*psum + b) in {-1, +1} (psum = 0.5 * x@sign(W).T)
                    nc.scalar.activation(hA[:, nb, ms], ps[:], SIGN,
                                         bias=bias_tiles[0][:, nb:nb + 1], scale=2.0)

            # ---------------- layers 2..n: exact +-1 x +-0.5 ----------------
            hin, hout = hA, hB
            for l in range(1, n_layers):
                last = l == n_layers - 1
                for nb in range(NB):
                    wt = wraw.tile([128, KO, 128], BF16, tag="w", name="wt")
                    nc.sync.dma_start(wt[:], ws[l][nb])
                    sw = wsgn.tile([128, KO, 128], FP8, tag="swhi", name="sw")
                    nc.vector.tensor_scalar(sw[:], wt[:], 0.0, 0.5, ALU.is_ge, ALU.subtract)
                    for mh in range(MH):
                        ms = slice(mh * MF, (mh + 1) * MF)
                        ps = psum.tile([128, MF], F32, tag="ps", name="ps")
                        if USE_DOUBLE_ROW and KO % 2 == 0:
                            for ko in range(0, KO, 2):
                                nc.tensor.matmul(
                                    ps[:], sw[:, ko:ko + 2, :], hin[:, ko:ko + 2, ms],
                                    start=(ko == 0), stop=(ko + 2 == KO),
                                    perf_mode=mybir.MatmulPerfMode.DoubleRow)
                        else:
                            for ko in range(KO):
                                nc.tensor.matmul(ps[:], sw[:, ko, :], hin[:, ko, ms],
                                                 start=(ko == 0), stop=(ko == KO - 1))
                        if last:
                            ot = xio.tile([128, MF], BF16, tag="ot", name="ot")
                            nc.scalar.activation(ot[:], ps[:], SIGN,
                                                 bias=bias_tiles[l][:, nb:nb + 1], scale=2.0)
                            nc.sync.dma_start(out[nb, :, ms], ot[:])
                        else:
                            nc.scalar.activation(hout[:, nb, ms], ps[:], SIGN,
                                                 bias=bias_tiles[l][:, nb:nb + 1], scale=2.0)
                hin, hout = hout, hin
    nc.compile()
    return nc


def _pack_w(W: np.ndarray) -> np.ndarray:
    """W [D, D] fp32 -> [NB, 128(p=k_in), KO, 128(n)] bf16 with
    WP[nb, p, ko, n] = bf16(W[nb*128 + n, ko*128 + p]).  Pure layout + a
    sign-preserving cast (bf16 keeps fp32's exponent range, so no value here
    can round across or onto zero)."""
    D = W.shape[0]
    nb = D // 128
    return np.ascontiguousarray(
        W.astype(ml_dtypes.bfloat16).reshape(nb, 128, nb, 128).transpose(0, 3, 2, 1)
    )


def _pack_b(b: np.ndarray) -> np.ndarray:
    return np.ascontiguousarray(b.astype(np.float32).reshape(-1, 128).T)


last_result = None  # BassKernelResults of the most recent run (for test.py)
_nc_cache = {}


def kernel(x, W1, b1, W2, b2, W3, b3, W4, b4):
    global last_result
    assert x.shape == (B_FULL, D_FULL)
    M = B_FULL // N_CORES

    if (D_FULL, M) not in _nc_cache:
        _nc_cache[(D_FULL, M)] = build_binary_mlp(D_FULL, M)
    nc = _nc_cache[(D_FULL, M)]

    xt = x.astype(np.float32).T  # [D, B]
    shared = {}
    for l, (W, b) in enumerate(((W1, b1), (W2, b2), (W3, b3), (W4, b4)), start=1):
        shared[f"w{l}"] = _pack_w(np.asarray(W))
        shared[f"b{l}"] = _pack_b(np.asarray(b))

    in_maps = []
    for c in range(N_CORES):
        m = dict(shared)
        m["xt"] = np.ascontiguousarray(xt[:, c * M:(c + 1) * M])
        in_maps.append(m)

    try:
        res = run_bass_kernel_spmd(nc, in_maps, core_ids=list(range(N_CORES)))
    except Exception:
        # one retry for transient device hiccups (NRT_EXEC_UNIT_UNRECOVERABLE
        # was observed once on an otherwise healthy worker)
        res = run_bass_kernel_spmd(nc, in_maps, core_ids=list(range(N_CORES)))
    last_result = res

    parts = []
    for c in range(N_CORES):
        o = np.asarray(res.results[c]["out"])  # [NB, 128, M] bf16, values +-1
        parts.append(o.reshape(D_FULL, M).T)   # -> [M, D] (rows are batch)
    return np.concatenate(parts, axis=0).astype(np.float32)


# revision 26
# speedup vs baseline: 1.2079x; 1.0017x over previous
"""Trainium2 Bass kernel for a 4-layer binary MLP (BinaryFCNN).

Reference computation (per layer):  h = sign_pm1(h @ sign_pm1(W).T + b)
with x: [8192, 4096] fp32, W_l: [4096, 4096] fp32, b_l: [4096] fp32.

Measured on 8 axon-tunneled TRN2 NeuronCores: HW exec time ~1.61 ms/core,
relative error 0.0092 vs the fp32 jax reference (a single borderline sign
flip in layer 1 out of 33.5M elements; binary nets amplify any flip, and
~1 flip is the irreducible level for any non-bit-identical fp32 matmul --
fp64 numpy vs jax-fp32-on-neuron measured 0 flips, so the reference itself
sits in that regime).

Strategy
--------
* Data-parallel over the batch: core c processes rows [c*1024, (c+1)*1024).
  No collectives; every core streams all four weight matrices (bf16, 33.5 MB
  per layer) -- fully hidden under the matmul stream.
* Activations live in SBUF feature-major ("h.T" layout, [feat, batch]): each
  layer's output is directly the next layer's moving operand; no transposes.
* All arithmetic on device. Host does value-preserving marshaling only:
  transpose/pack, fp32->bf16 cast of W (sign-preserving at these magnitudes),
  final bf16 -> fp32 cast of the +-1 output.
* Weights are encoded on device as (w >= 0) - 0.5 in {-0.5, +0.5} (one DVE
  op). +-0.5 is a power of two so products/partial sums stay exact; the sign
  activation uses ACT Sign(2*psum + b) == sign(h @ sign(W).T + b), with the
  per-feature bias as the ACT per-partition bias operand -- one ACT op per
  PSUM tile, reading PSUM and writing the next h tile.
* Layers 2..4 are bit-exact: +-1 activations and +-0.5 weights in fp8e4m3
  with fp32 PSUM accumulation (half-integer sums << 2^24). fp8 DoubleRow
  (perf_mode) pairs k-chunks for 2 fp8 MACs/PE/cycle -- these layers run at
  2x bf16 rate (~218 us each).
* Layer 1 is the only inexact matmul. x is split on device into two fp16
  digits: hi = fp16(x), lo = fp16((x - hi) * 2^11) (scaling dodges fp16
  subnormal flush; the lo pass uses weights pre-scaled by 2^-11, an exact
  power-of-two). Both digit passes accumulate into one PSUM tile. fp16 is
  the optimal digit dtype: 11 mantissa bits per PE pass vs 8 (bf16) or
  2x4 (fp8 DoubleRow pair).
* A short burst of throwaway matmuls during the x-prep prologue releases the
  PE HAM clock gate (cold 1.2 GHz -> warm 2.4 GHz) before the real stream.

Per-core floor: layer 1 = 2 fp16 passes = 874 us, layers 2-4 = 3 DR passes
= 654 us; measured 1.61 ms = floor + ~5% (prologue, mh boundary, tail drain).
"""
import numpy as np
import ml_dtypes

import concourse.bass as bass
import concourse.tile as tile
from concourse import bacc
import concourse.mybir as mybir
from concourse.bass_utils import run_bass_kernel_spmd

F32 = mybir.dt.float32
F16 = mybir.dt.float16
BF16 = mybir.dt.bfloat16
FP8 = mybir.dt.float8e4
ALU = mybir.AluOpType
SIGN = mybir.ActivationFunctionType.Sign

N_CORES = 8
D_FULL = 4096
B_FULL = 8192
MF = 512  # matmul moving free dim == one fp32 PSUM bank

# fp8 DoubleRow for layers 2..4: 2 fp8 MACs/PE/cycle (measured ~1.4x at FD=512),
# and halves the matmul instruction count. Exactness is preserved (+-1 x +-0.5
# products, fp32 accumulate).
USE_DOUBLE_ROW = True


def build_binary_mlp(D: int, M: int, n_layers: int = 4) -> bass.Bass:
    """Emit the per-core kernel. D = feature dim, M = per-core batch rows."""
    KO = D // 128  # contraction chunks (also input-feature blocks)
    NB = D // 128  # output-feature blocks
    MF = min(512, M)  # moving free dim (one fp32 PSUM bank at 512)
    MH = M // MF   # batch slices of the moving operand

    # Bacc (not raw Bass) + the trailing compile() pass: Bacc's compile
    # legalizes things raw Bass output trips over in walrus (e.g. the
    # 1-sync-wait-per-instruction cap).
    nc = bacc.Bacc("TRN2", target_bir_lowering=False, debug=False)
    xt = nc.declare_dram_parameter("xt", [D, M], F32, isOutput=False)
    ws = [
        nc.declare_dram_parameter(f"w{l + 1}", [NB, 128, KO, 128], BF16, isOutput=False)
        for l in range(n_layers)
    ]
    bs = [
        nc.declare_dram_parameter(f"b{l + 1}", [128, NB], F32, isOutput=False)
        for l in range(n_layers)
    ]
    out = nc.declare_dram_parameter("out", [NB, 128, M], BF16, isOutput=True)

    with tile.TileContext(nc) as tc:
        with (
            tc.tile_pool(name="const", bufs=1) as const,
            tc.tile_pool(name="wraw", bufs=3) as wraw,
            tc.tile_pool(name="wsgn", bufs=2) as wsgn,
            tc.tile_pool(name="xio", bufs=4) as xio,
            tc.tile_pool(name="psum", bufs=5, space="PSUM") as psum,
            tc.tile_pool(name="psum1", bufs=1, space="PSUM") as psum1,
        ):
            bias_tiles = []
            for l in range(n_layers):
                bt = const.tile([128, NB], F32, tag=f"bias{l}", name=f"bias{l}")
                nc.sync.dma_start(bt[:], bs[l][:])
                bias_tiles.append(bt)

            # PE warm-up: the first real matmul waits ~20us for the x digit
            # prep; fill that window with throwaway matmuls so the HAM clock
            # gate (cold 1.2 GHz -> warm 2.4 GHz after ~3.4us of activity) is
            # already released when the real stream starts.
            if M >= 512:
                wu = const.tile([128, MF], F16, tag="warm", name="warm")
                nc.vector.memset(wu[:], 1.0)
                wps = psum1.tile([128, MF], F32, tag="wps", name="wps")
                n_wu = 72
                for i in range(n_wu):
                    nc.tensor.matmul(wps[:], wu[:, :128], wu[:],
                                     start=(i == 0), stop=(i == n_wu - 1))

            # ping-pong activation buffers, feature-major, +-1 in fp8
            hA = const.tile([128, KO, M], FP8, tag="hA", name="hA")
            hB = const.tile([128, KO, M], FP8, tag="hB", name="hB")

            # ---------------- layer 1: fp16 hi/lo digit passes ----------------
            for mh in range(MH):  # noqa: B007
                ms = slice(mh * MF, (mh + 1) * MF)
                hi = const.tile([128, KO, MF], F16, tag="hi", name="hi")
                lo = const.tile([128, KO, MF], F16, tag="lo", name="lo")
                for ko in range(KO):
                    xc = xio.tile([128, MF], F32, tag="xc", name="xc")
                    nc.sync.dma_start(xc[:], xt[ko * 128:(ko + 1) * 128, ms])
                    nc.scalar.copy(hi[:, ko, :], xc[:])  # fp16 round-to-nearest
                    r = xio.tile([128, MF], F32, tag="r", name="r")
                    nc.vector.tensor_sub(r[:], xc[:], hi[:, ko, :])
                    # scale residual by 2^11 so fp16 never goes subnormal
                    nc.vector.tensor_scalar_mul(lo[:, ko, :], r[:], 2048.0)
                for nb in range(NB):
                    wt = wraw.tile([128, KO, 128], BF16, tag="w", name="wt")
                    nc.sync.dma_start(wt[:], ws[0][nb])
                    sw_hi = wsgn.tile([128, KO, 128], F16, tag="swhi", name="sw_hi")
                    nc.vector.tensor_scalar(sw_hi[:], wt[:], 0.0, 0.5, ALU.is_ge, ALU.subtract)
                    sw_lo = wsgn.tile([128, KO, 128], F16, tag="swlo", name="sw_lo")
                    nc.vector.tensor_scalar_mul(sw_lo[:], sw_hi[:], 2.0 ** -11)
                    ps = psum.tile([128, MF], F32, tag="ps", name="ps")
                    for ko in range(KO):
                        nc.tensor.matmul(ps[:], sw_hi[:, ko, :], hi[:, ko, :],
                                         start=(ko == 0), stop=False)
                        nc.tensor.matmul(ps[:], sw_lo[:, ko, :], lo[:, ko, :],
                                         start=False, stop=(ko == KO - 1))
                    # h1 = Sign(2*psum + b) in {-1, +1} (psum = 0.5 * x@sign(W).T)
                    nc.scalar.activation(hA[:, nb, ms], ps[:], SIGN,
                                         bias=bias_tiles[0][:, nb:nb + 1], scale=2.0)

            # ---------------- layers 2..n: exact +-1 x +-0.5 ----------------
            hin, hout = hA, hB
            for l in range(1, n_layers):
                last = l == n_layers - 1
                for nb in range(NB):
                    wt = wraw.tile([128, KO, 128], BF16, tag="w", name="wt")
                    nc.sync.dma_start(wt[:], ws[l][nb])
                    sw = wsgn.tile([128, KO, 128], FP8, tag="swhi", name="sw")
                    nc.vector.tensor_scalar(sw[:], wt[:], 0.0, 0.5, ALU.is_ge, ALU.subtract)
                    for mh in range(MH):
                        ms = slice(mh * MF, (mh + 1) * MF)
                        ps = psum.tile([128, MF], F32, tag="ps", name="ps")
                        if USE_DOUBLE_ROW and KO % 2 == 0:
                            for ko in range(0, KO, 2):
                                nc.tensor.matmul(
                                    ps[:], sw[:, ko:ko + 2, :], hin[:, ko:ko + 2, ms],
                                    start=(ko == 0), stop=(ko + 2 == KO),
                                    perf_mode=mybir.MatmulPerfMode.DoubleRow)
                        else:
                            for ko in range(KO):
                                nc.tensor.matmul(ps[:], sw[:, ko, :], hin[:, ko, ms],
                                                 start=(ko == 0), stop=(ko == KO - 1))
                        if last:
                            ot = xio.tile([128, MF], BF16, tag="ot", name="ot")
                            nc.scalar.activation(ot[:], ps[:], SIGN,
                                                 bias=bias_tiles[l][:, nb:nb + 1], scale=2.0)
                            nc.sync.dma_start(out[nb, :, ms], ot[:])
                        else:
                            nc.scalar.activation(hout[:, nb, ms], ps[:], SIGN,
                                                 bias=bias_tiles[l][:, nb:nb + 1], scale=2.0)
                hin, hout = hout, hin
    nc.compile()
    return nc


def _pack_w(W: np.ndarray) -> np.ndarray:
    """W [D, D] fp32 -> [NB, 128(p=k_in), KO, 128(n)] bf16 with
    WP[nb, p, ko, n] = bf16(W[nb*128 + n, ko*128 + p]).  Pure layout + a
    sign-preserving cast (bf16 keeps fp32's exponent range, so no value here
    can round across or onto zero)."""
    D = W.shape[0]
    nb = D // 128
    return np.ascontiguousarray(
        W.astype(ml_dtypes.bfloat16).reshape(nb, 128, nb, 128).transpose(0, 3, 2, 1)
    )


def _pack_b(b: np.ndarray) -> np.ndarray:
    return np.ascontiguousarray(b.astype(np.float32).reshape(-1, 128).T)


last_result = None  # BassKernelResults of the most recent run (for test.py)
_nc_cache = {}


def kernel(x, W1, b1, W2, b2, W3, b3, W4, b4):
    global last_result
    assert x.shape == (B_FULL, D_FULL)
    M = B_FULL // N_CORES

    if (D_FULL, M) not in _nc_cache:
        _nc_cache[(D_FULL, M)] = build_binary_mlp(D_FULL, M)
    nc = _nc_cache[(D_FULL, M)]

    xt = x.astype(np.float32).T  # [D, B]
    shared = {}
    for l, (W, b) in enumerate(((W1, b1), (W2, b2), (W3, b3), (W4, b4)), start=1):
        shared[f"w{l}"] = _pack_w(np.asarray(W))
        shared[f"b{l}"] = _pack_b(np.asarray(b))

    in_maps = []
    for c in range(N_CORES):
        m = dict(shared)
        m["xt"] = np.ascontiguousarray(xt[:, c * M:(c + 1) * M])
        in_maps.append(m)

    try:
        res = run_bass_kernel_spmd(nc, in_maps, core_ids=list(range(N_CORES)))
    except Exception:
        # one retry for transient device hiccups (NRT_EXEC_UNIT_UNRECOVERABLE
        # was observed once on an otherwise healthy worker)
        res = run_bass_kernel_spmd(nc, in_maps, core_ids=list(range(N_CORES)))
    last_result = res

    parts = []
    for c in range(N_CORES):
        o = np.asarray(res.results[c]["out"])  # [NB, 128, M] bf16, values +-1
        parts.append(o.reshape(D_FULL, M).T)   # -> [M, D] (rows are batch)
    return np.concatenate(parts, axis=0).astype(np.float32)
